# revision 1
# baseline (speedup 1.0000x reference)
"""Trainium2 Bass kernel for AdaptiveGatedSLNNStep.

Reference computation (B=4096, D=1024, R=2048, MAXR=4096):
    x  = inputs[:, 0, :]                  # [B, D]
    h  = state[:, 0, :R]                  # [B, R]
    ip = x @ Wi[:R, :].T                  # [B, R]
    rp = h @ Wr[:R, :R]                   # [B, R]
    g  = sigmoid(x @ Wg[:3R, :].T)        # [B, 3R] -> i, f, o
    ns = 0.9*(f*h) + 0.1*tanh(i*(ip+rp))
    ns = o * ns
    ns = where(ns > 0.5, ns - 0.5, ns)
    out = pad(ns, [B, 1, MAXR])

Sharding: 8 cores = 4 batch groups x 2 reservoir-column groups; no
collectives (output blocks are disjoint). Each core computes its
[1024, 1024] block of ns in FEATURE-MAJOR layout: out[r, b].

Per core: x^T [D, BC] and h^T [R, BC] are resident in SBUF as the
moving matmul operands; weights stream per 128-row reservoir block as
the stationary (lhsT) operand. Matmuls run as float32r (full-rate
fp32), N=512 batch columns per instruction, accumulating in PSUM.

h^T rows are permuted on the host so this core's own RC-slice comes
first: the elementwise f*h term then reads resident h^T tile j
directly (identical SPMD graph on every core), and wr's contraction
rows are permuted identically so the reservoir matmul is unchanged.
The host transposes per-core outputs while assembling the padded
[B, 1, MAXR] result.
"""

import numpy as np

import concourse.bass as bass
import concourse.mybir as mybir
import concourse.tile as tile
from concourse import bacc
from concourse.bass import ds
from concourse.bass_utils import run_bass_kernel_spmd

F32 = mybir.dt.float32
F32R = mybir.dt.float32r
AF = mybir.ActivationFunctionType
ALU = mybir.AluOpType

B = 4096          # global batch
D = 1024          # input dim
R = 2048          # reservoir dim
MAXR = 4096       # padded reservoir dim
NCORES = 8
DGROUPS = 4       # batch groups
RGROUPS = 2       # reservoir column groups
BC = B // DGROUPS     # 1024 batch rows per core
RC = R // RGROUPS     # 1024 reservoir rows (output features) per core
P = 128               # partitions
NJ = RC // P          # 8 reservoir row blocks per core
NBH = 512             # batch columns per matmul (moving operand)
NH = BC // NBH        # 2 batch halves
KD = D // P           # 8  contraction tiles over D
KR = R // P           # 16 contraction tiles over R

_cache = {}


def _tf32(a):
    """Round fp32 to tfloat32 (10 mantissa bits, round-to-nearest-even).

    The PE's float32r mode computes with tf32-truncated operands; rounding
    on the host makes the data a fixpoint of the hardware rounding so the
    kernel is deterministic and matches the emulated error model.
    """
    a = np.ascontiguousarray(a, dtype=np.float32)
    u = a.view(np.uint32).copy()
    half = np.uint32(1 << 12)
    lsb = (u >> np.uint32(13)) & np.uint32(1)
    u += half - np.uint32(1) + lsb
    u &= np.uint32(0xFFFFE000)
    return u.view(np.float32)


def _build():
    nc = bacc.Bacc("TRN2", target_bir_lowering=False, debug=False,
                   num_devices=NCORES)

    xT_d = nc.dram_tensor("xT", [D, BC], F32R, kind="ExternalInput")
    hT_d = nc.dram_tensor("hT", [R, BC], F32R, kind="ExternalInput")
    # weights, pre-packed on host per reservoir block in partition-major
    # layout so every weight DMA reads 4KB-contiguous runs per partition
    # (512B runs don't saturate the SDMA bus width on silicon):
    # wi[j, p, k, c] = Wi[rg*RC + j*P + c, k*P + p], etc. (see _shard)
    wi_d = nc.dram_tensor("wi", [NJ, P, KD, P], F32R, kind="ExternalInput")
    wr_d = nc.dram_tensor("wr", [NJ, P, KR, P], F32R, kind="ExternalInput")
    wg_d = nc.dram_tensor("wg", [NJ, 3, P, KD, P], F32R,
                          kind="ExternalInput")
    out_d = nc.dram_tensor("out", [RC, BC], F32, kind="ExternalOutput")

    with tile.TileContext(nc) as tc:
        with (
            tc.tile_pool(name="acts", bufs=1) as acts,
            tc.tile_pool(name="wts", bufs=3) as wts,
            tc.tile_pool(name="wrp", bufs=2) as wrp,
            tc.tile_pool(name="ew", bufs=6) as ew,
            tc.tile_pool(name="vpool", bufs=2) as vpool,
            tc.tile_pool(name="psum", bufs=2, space="PSUM") as psum,
            tc.tile_pool(name="psum_pre", bufs=6, space="PSUM") as psum_pre,
        ):
            # resident moving operands
            xt = acts.tile([P, KD, BC], F32R, tag="xt")
            ht = acts.tile([P, KR, BC], F32R, tag="ht")

            # PE clock warmup: the HAM clock gate only releases the full
            # 2.4 GHz after ~3.4us of sustained PE activity, and the
            # first real matmul can't start until its weights arrive
            # (~4us of DMA). Burn that window on dummy matmuls over a
            # zeroed tile so the real matmuls start at full rate.
            warm = acts.tile([P, NBH], F32, tag="warm")
            nc.gpsimd.memset(warm[:], 0.0)
            wpsum = psum.tile([P, NBH], F32, tag="gate", name="warmp")
            for w in range(2):
                nc.tensor.matmul(wpsum[:], warm[:, 0:P], warm[:],
                                 start=(w == 0), stop=(w == 1))
            nc.scalar.activation(warm[:, 0:P], wpsum[:, 0:P], AF.Copy,
                                 scale=0.0)

            wgs, wis, wrs = [], [], []

            def load_wgwi(j):
                wg = wts.tile([P, 3, KD, P], F32R, tag="wg")
                for g in range(3):
                    nc.sync.dma_start(wg[:, g, :, :], wg_d[j, g])
                wi = wts.tile([P, KD, P], F32R, tag="wi")
                nc.sync.dma_start(wi[:], wi_d[j])
                wgs.append(wg)
                wis.append(wi)

            def load_wr(j):
                wr = wrp.tile([P, KR, P], F32R, tag="wr")
                nc.sync.dma_start(wr[:], wr_d[j])
                wrs.append(wr)

            # Phase A of unit (j, h): three gate matmul groups, each
            # drained to SBUF by a sigmoid immediately (2 rotating PSUM
            # banks), plus the input-part matmuls left OPEN in a pre
            # bank. Only needs x^T + wg_j + wi_j.
            sig_tiles = {}
            pre_tiles = {}

            def emit_A(j, paired=True):
                # paired: both batch halves interleaved at k so each
                # stationary weight tile feeds two consecutive matmuls
                # (amortizes the PE weight load). Block 0 instead runs
                # h-outer so it keeps pace with the x^T half-tile
                # stream during startup.
                wg, wi = wgs[j], wis[j]
                bss = [ds(h * NBH, NBH) for h in range(NH)]
                sigs_h = [[], []]
                pres = [psum_pre.tile([P, NBH], F32, tag="pre",
                                      name=f"pre{j}h{h}")
                        for h in range(NH)]
                if paired:
                    for g, stag in enumerate(("si", "sf", "so")):
                        gps = [psum.tile([P, NBH], F32, tag="gate",
                                         name=f"gp{g}h{h}")
                               for h in range(NH)]
                        for k in range(KD):
                            for h in range(NH):
                                nc.tensor.matmul(gps[h][:], wg[:, g, k, :],
                                                 xt[:, k, bss[h]],
                                                 start=(k == 0),
                                                 stop=(k == KD - 1))
                        for h in range(NH):
                            s = ew.tile([P, NBH], F32, tag=stag,
                                        name=f"s{g}h{h}")
                            nc.scalar.activation(s[:], gps[h][:], AF.Sigmoid)
                            sigs_h[h].append(s)
                    for k in range(KD):
                        for h in range(NH):
                            nc.tensor.matmul(pres[h][:], wi[:, k, :],
                                             xt[:, k, bss[h]],
                                             start=(k == 0), stop=False)
                else:
                    # startup variant: per half, run (g0, g1) k-outer in
                    # the two gate banks, then (g2, pre) k-outer - two
                    # matmuls per arriving x^T half-tile
                    for h in range(NH):
                        g0 = psum.tile([P, NBH], F32, tag="gate",
                                       name=f"g0h{h}")
                        g1 = psum.tile([P, NBH], F32, tag="gate",
                                       name=f"g1h{h}")
                        for k in range(KD):
                            nc.tensor.matmul(g0[:], wg[:, 0, k, :],
                                             xt[:, k, bss[h]],
                                             start=(k == 0),
                                             stop=(k == KD - 1))
                            nc.tensor.matmul(g1[:], wg[:, 1, k, :],
                                             xt[:, k, bss[h]],
                                             start=(k == 0),
                                             stop=(k == KD - 1))
                        s0 = ew.tile([P, NBH], F32, tag="si",
                                     name=f"sA0h{h}")
                        nc.scalar.activation(s0[:], g0[:], AF.Sigmoid)
                        s1 = ew.tile([P, NBH], F32, tag="sf",
                                     name=f"sA1h{h}")
                        nc.scalar.activation(s1[:], g1[:], AF.Sigmoid)
                        g2 = psum.tile([P, NBH], F32, tag="gate",
                                       name=f"g2h{h}")
                        for k in range(KD):
                            nc.tensor.matmul(g2[:], wg[:, 2, k, :],
                                             xt[:, k, bss[h]],
                                             start=(k == 0),
                                             stop=(k == KD - 1))
                            nc.tensor.matmul(pres[h][:], wi[:, k, :],
                                             xt[:, k, bss[h]],
                                             start=(k == 0), stop=False)
                        s2 = ew.tile([P, NBH], F32, tag="so",
                                     name=f"sA2h{h}")
                        nc.scalar.activation(s2[:], g2[:], AF.Sigmoid)
                        sigs_h[h] += [s0, s1, s2]
                for h in range(NH):
                    # fold 9*(sigmoid(f)*h) here - off the B critical path
                    sf_t = sigs_h[h][1]
                    nc.vector.tensor_tensor(sf_t[:], sf_t[:],
                                            ht[:, j, bss[h]].bitcast(F32),
                                            op=ALU.mult)
                    nc.vector.tensor_scalar(sf_t[:], sf_t[:], 9.0, None,
                                            op0=ALU.mult)
                for h in range(NH):
                    sig_tiles[(j, h)] = sigs_h[h]
                    pre_tiles[(j, h)] = pres[h]

            # Phase B of unit (j, h): finish the pre accumulation with
            # the reservoir part (needs full h^T + wr_j), then the
            # elementwise epilogue and the output DMA.
            def emit_B(j):
                wr = wrs[j]
                bss = [ds(h * NBH, NBH) for h in range(NH)]
                pres = [pre_tiles.pop((j, h)) for h in range(NH)]
                for h in range(NH):
                    for k in range(KR):
                        nc.tensor.matmul(pres[h][:], wr[:, k, :],
                                         ht[:, k, bss[h]],
                                         start=False, stop=(k == KR - 1))
                for h in range(NH):
                    bs = bss[h]
                    si, sf, so = sig_tiles.pop((j, h))
                    pre = pres[h]
                    m = ew.tile([P, NBH], F32, tag="sf", name=f"m{h}")
                    v = vpool.tile([P, NBH], F32, tag="v", name=f"v{h}")
                    # the very last unit runs its epilogue in two column
                    # chunks so the DVE/ACT chains pipeline into the
                    # kernel tail instead of serializing after it
                    last = (j == NJ - 1 and h == NH - 1)
                    chunks = ([ds(0, NBH // 2), ds(NBH // 2, NBH // 2)]
                              if last else [ds(0, NBH)])
                    for cs in chunks:
                        # si <- tanh(si * pre)
                        nc.vector.tensor_tensor(si[:, cs], si[:, cs],
                                                pre[:, cs], op=ALU.mult)
                        nc.scalar.activation(si[:, cs], si[:, cs], AF.Tanh)
                        # sf already holds 9*(sigmoid(f)*h) from phase A
                        nc.vector.tensor_tensor(sf[:, cs], sf[:, cs],
                                                si[:, cs], op=ALU.add)
                        # so <- po = so*(9fh + t); new_state = 0.1*po
                        nc.vector.tensor_tensor(so[:, cs], so[:, cs],
                                                sf[:, cs], op=ALU.mult)
                        # spike threshold: v = 0.1*po; v>0.5 -> v-0.5
                        nc.vector.tensor_scalar(m[:, cs], so[:, cs], 5.0,
                                                0.5, op0=ALU.is_gt,
                                                op1=ALU.mult)
                        nc.scalar.activation(v[:, cs], so[:, cs], AF.Copy,
                                             scale=0.1)
                        nc.vector.tensor_tensor(v[:, cs], v[:, cs],
                                                m[:, cs], op=ALU.subtract)
                    if last:
                        half = NBH // 2
                        for c in range(2):
                            nc.sync.dma_start(
                                out_d[j * P:(j + 1) * P,
                                      h * NBH + c * half:
                                      h * NBH + (c + 1) * half],
                                v[:, ds(c * half, half)])
                    else:
                        nc.gpsimd.dma_start(
                            out_d[j * P:(j + 1) * P,
                                  h * NBH:(h + 1) * NBH], v[:])

            # DMA order: wg0+wi0, x^T (first gates start ~6us in),
            # then h^T with the next blocks' gate weights interleaved
            # and wr0 near the tail so B0 is never the head blocker.
            wg0 = wts.tile([P, 3, KD, P], F32R, tag="wg")
            wi0 = wts.tile([P, KD, P], F32R, tag="wi")
            # first gate matmul needs only wg0[g0, k0] (64 KiB) + the
            # h0 half of xt k0; x^T and h^T stream per batch half (h0
            # first) so block 0's h-outer phase A keeps pace
            nc.sync.dma_start(wg0[:, 0, 0, :], wg_d[0, 0, :, 0, :])
            nc.sync.dma_start(xt[:, 0, 0:NBH], xT_d[0:P, 0:NBH])
            nc.sync.dma_start(wg0[:, 0, 1:, :], wg_d[0, 0, :, 1:, :])
            nc.sync.dma_start(wg0[:, 1, :, :], wg_d[0, 1])
            wgs.append(wg0)
            wis.append(wi0)
            for k in range(1, KD):
                nc.sync.dma_start(xt[:, k, 0:NBH],
                                  xT_d[k * P:(k + 1) * P, 0:NBH])
                if k == 2:
                    nc.sync.dma_start(wg0[:, 2, :, :], wg_d[0, 2])
                if k == 4:
                    nc.sync.dma_start(wi0[:], wi_d[0])
            for k in range(KD):
                nc.sync.dma_start(xt[:, k, NBH:BC],
                                  xT_d[k * P:(k + 1) * P, NBH:BC])
            for k in range(KR):
                nc.sync.dma_start(ht[:, k, 0:NBH],
                                  hT_d[k * P:(k + 1) * P, 0:NBH])
                if k == 3:
                    load_wgwi(1)
                if k == 9:
                    load_wgwi(2)
                if k == 10:
                    load_wr(0)
            for k in range(KR):
                nc.sync.dma_start(ht[:, k, NBH:BC],
                                  hT_d[k * P:(k + 1) * P, NBH:BC])

            emit_A(0, paired=False)
            emit_A(1)
            for j in range(NJ):
                if j + 1 < NJ:
                    load_wr(j + 1)
                if j + 3 < NJ:
                    load_wgwi(j + 3)
                if j + 2 < NJ:
                    emit_A(j + 2)
                emit_B(j)

    nc.compile()
    return nc


def _shard(inputs, state, reservoir_weights, input_weights, gate_weights):
    x = np.ascontiguousarray(inputs[:, 0, :], dtype=np.float32)
    h = np.ascontiguousarray(state[:, 0, :R], dtype=np.float32)

    # weight prep depends only on the column group (2 variants across 8
    # cores) - compute once per group and share the arrays
    wsets = {}
    for rg in range(RGROUPS):
        rsl = slice(rg * RC, (rg + 1) * RC)
        osl = slice((1 - rg) * RC, (1 - rg) * RC + RC)  # other half
        wr = np.concatenate([reservoir_weights[rsl, rsl.start:rsl.stop],
                             reservoir_weights[osl, rsl.start:rsl.stop]],
                            axis=0)
        wg = np.stack(
            [gate_weights[g * R + rg * RC:g * R + rg * RC + RC, :].T
             for g in range(3)], axis=1)
        wi = np.asarray(input_weights[rsl, :]).T  # [D, RC]
        wsets[rg] = {
            "wi": _tf32(wi.reshape(KD, P, NJ, P).transpose(2, 1, 0, 3)),
            "wr": _tf32(np.asarray(wr).reshape(KR, P, NJ, P)
                        .transpose(2, 1, 0, 3)),
            "wg": _tf32(np.asarray(wg).reshape(KD, P, 3, NJ, P)
                        .transpose(3, 2, 1, 0, 4)),
        }

    in_maps = []
    for core in range(NCORES):
        d, rg = divmod(core, RGROUPS)
        bsl = slice(d * BC, (d + 1) * BC)
        rsl = slice(rg * RC, (rg + 1) * RC)
        osl = slice((1 - rg) * RC, (1 - rg) * RC + RC)
        hT = np.concatenate([h[bsl, rsl].T, h[bsl, osl].T], axis=0)
        in_maps.append({
            "xT": _tf32(x[bsl].T),
            "hT": _tf32(hT),
            **wsets[rg],
        })
    return in_maps


def _run(inputs, state, reservoir_weights, input_weights, gate_weights,
         trace=False):
    if "nc" not in _cache:
        _cache["nc"] = _build()
    nc = _cache["nc"]
    in_maps = _shard(inputs, state, reservoir_weights, input_weights,
                     gate_weights)
    res = run_bass_kernel_spmd(nc, in_maps, core_ids=list(range(NCORES)),
                               trace=trace)
    out = np.zeros((B, 1, MAXR), dtype=np.float32)
    for core in range(NCORES):
        d, rg = divmod(core, RGROUPS)
        out[d * BC:(d + 1) * BC, 0, rg * RC:(rg + 1) * RC] = \
            res.results[core]["out"].T
    return out, res


def kernel(inputs, state, reservoir_weights, input_weights, gate_weights):
    out, _ = _run(inputs, state, reservoir_weights, input_weights,
                  gate_weights)
    return out



# revision 3
# speedup vs baseline: 1.2924x; 1.2924x over previous
"""Trainium2 Bass kernel for AdaptiveGatedSLNNStep.

Reference computation (B=4096, D=1024, R=2048, MAXR=4096):
    x  = inputs[:, 0, :]                  # [B, D]
    h  = state[:, 0, :R]                  # [B, R]
    ip = x @ Wi[:R, :].T                  # [B, R]
    rp = h @ Wr[:R, :R]                   # [B, R]
    g  = sigmoid(x @ Wg[:3R, :].T)        # [B, 3R] -> i, f, o
    ns = 0.9*(f*h) + 0.1*tanh(i*(ip+rp))
    ns = o * ns
    ns = where(ns > 0.5, ns - 0.5, ns)
    out = pad(ns, [B, 1, MAXR])

Sharding: 8 cores = 4 batch groups x 2 reservoir-column groups; no
collectives (output blocks are disjoint). Each core computes its
[1024, 1024] block of ns in FEATURE-MAJOR layout: out[r, b].

All matmuls run as fp8 e4m3 in DoubleRow perf mode (two contraction
rows per PE pass), with a 3-pass residual-correction scheme that
recovers ~bf16 accuracy at 0.75x the fp32r cycle count:
    A@B ~= Q(A)Q(B) + Q(dA)Q(B) + Q(A)Q(dB),   dA = A - Q(A)
Operands are pre-scaled by powers of two on the host (x,h by 16,
weights by 512) so every pass shares one PSUM scale (8192), folded
into the sigmoid/tanh activation scale for free. The elementwise
f*h term reads a separate exact fp32 copy of h (fp8 h there would
dominate the error via spike-threshold flips).

h^T rows are permuted on the host so this core's own RC-slice comes
first; wr's contraction rows are permuted identically. The host
transposes per-core outputs while assembling the padded result.
"""

import numpy as np
import ml_dtypes

import concourse.bass as bass
import concourse.mybir as mybir
import concourse.tile as tile
from concourse import bacc
from concourse.bass import ds
from concourse.bass_utils import run_bass_kernel_spmd

F32 = mybir.dt.float32
F8 = mybir.dt.float8e4
NP8 = ml_dtypes.float8_e4m3
AF = mybir.ActivationFunctionType
ALU = mybir.AluOpType
PM = mybir.MatmulPerfMode

B = 4096          # global batch
D = 1024          # input dim
R = 2048          # reservoir dim
MAXR = 4096       # padded reservoir dim
NCORES = 8
DGROUPS = 4       # batch groups
RGROUPS = 2       # reservoir column groups
BC = B // DGROUPS     # 1024 batch rows per core
RC = R // RGROUPS     # 1024 reservoir rows (output features) per core
P = 128               # partitions
NJ = RC // P          # 8 reservoir row blocks per core
NBH = 512             # batch columns per matmul (moving operand)
NH = BC // NBH        # 2 batch halves
KD = D // P           # 8  contraction tiles over D
KD2 = KD // 2         # 4  DoubleRow k-pairs over D
KR = R // P           # 16 contraction tiles over R
KR2 = KR // 2         # 8  DoubleRow k-pairs over R

SX = np.float32(16.0)    # x/h fp8 pre-scale
SW = np.float32(512.0)   # weight fp8 pre-scale
INV_PSUM = float(1.0 / (float(SX) * float(SW)))   # 1/8192

_cache = {}


def _build():
    nc = bacc.Bacc("TRN2", target_bir_lowering=False, debug=False,
                   num_devices=NCORES)

    # q/d pairs: main fp8 quantization and its fp8-quantized residual
    xq_d = nc.dram_tensor("xq", [P, KD, BC], F8, kind="ExternalInput")
    xd_d = nc.dram_tensor("xd", [P, KD, BC], F8, kind="ExternalInput")
    hq_d = nc.dram_tensor("hq", [P, KR, BC], F8, kind="ExternalInput")
    hd_d = nc.dram_tensor("hd", [P, KR, BC], F8, kind="ExternalInput")
    # exact h (own half, feature-major) for the elementwise f*h term
    hf_d = nc.dram_tensor("hf", [P, NJ, BC], F32, kind="ExternalInput")
    # weights packed per reservoir block, partition-major, with the q/d
    # variants adjacent and contraction pre-grouped into DoubleRow pairs:
    # wg[j, p, g, qd, t, i, m] = Wsc[(2t+i)*128+p, j*128+m]
    wg_d = nc.dram_tensor("wg", [NJ, P, 3, 2, KD2, 2, P], F8,
                          kind="ExternalInput")
    wi_d = nc.dram_tensor("wi", [NJ, P, 2, KD2, 2, P], F8,
                          kind="ExternalInput")
    wr_d = nc.dram_tensor("wr", [NJ, P, 2, KR2, 2, P], F8,
                          kind="ExternalInput")
    out_d = nc.dram_tensor("out", [RC, BC], F32, kind="ExternalOutput")

    with tile.TileContext(nc) as tc:
        with (
            tc.tile_pool(name="acts", bufs=1) as acts,
            tc.tile_pool(name="wts", bufs=3) as wts,
            tc.tile_pool(name="wrp", bufs=2) as wrp,
            tc.tile_pool(name="ew", bufs=6) as ew,
            tc.tile_pool(name="vpool", bufs=2) as vpool,
            tc.tile_pool(name="psum", bufs=2, space="PSUM") as psum,
            tc.tile_pool(name="psum_pre", bufs=6, space="PSUM") as psum_pre,
        ):
            # resident moving operands
            xq = acts.tile([P, KD, BC], F8, tag="xq")
            xd = acts.tile([P, KD, BC], F8, tag="xd")
            hq = acts.tile([P, KR, BC], F8, tag="hq")
            hd = acts.tile([P, KR, BC], F8, tag="hd")
            hfv = acts.tile([P, NJ, BC], F32, tag="hf")

            # PE clock warmup: the clock gate only releases full rate
            # after ~3us of sustained PE activity, and the first real
            # matmul can't start until its weights arrive. Burn that
            # window on dummy matmuls over a zeroed tile.
            warm = acts.tile([P, NBH], F32, tag="warm")
            nc.gpsimd.memset(warm[:], 0.0)
            wpsum = psum.tile([P, NBH], F32, tag="gate", name="warmp")
            for w in range(2):
                nc.tensor.matmul(wpsum[:], warm[:, 0:P], warm[:],
                                 start=(w == 0), stop=(w == 1))
            nc.scalar.activation(warm[:, 0:P], wpsum[:, 0:P], AF.Copy,
                                 scale=0.0)

            wgs, wis, wrs = [], [], []

            def load_wgwi(j):
                wg = wts.tile([P, 3, 2, KD2, 2, P], F8, tag="wg")
                nc.sync.dma_start(wg[:], wg_d[j])
                wi = wts.tile([P, 2, KD2, 2, P], F8, tag="wi")
                nc.sync.dma_start(wi[:], wi_d[j])
                wgs.append(wg)
                wis.append(wi)

            def load_wr(j):
                wr = wrp.tile([P, 2, KR2, 2, P], F8, tag="wr")
                nc.sync.dma_start(wr[:], wr_d[j])
                wrs.append(wr)

            # The three correction passes per matmul group: (moving
            # operand, qd index of the stationary operand). xd last so
            # startup DMA has extra slack for the residual tensors.
            def gate_passes(j):
                wg = wgs[j]
                return lambda g: [(xq, wg[:, g, 0]), (xq, wg[:, g, 1]),
                                  (xd, wg[:, g, 0])]

            # Phase A of unit (j, h): three gate matmul groups, each
            # drained to SBUF by a sigmoid immediately (2 rotating PSUM
            # banks), plus the input-part matmuls left OPEN in a pre
            # bank. Only needs x + wg_j + wi_j.
            sig_tiles = {}
            pre_tiles = {}

            def emit_A(j, paired=True):
                wg, wi = wgs[j], wis[j]
                gp = gate_passes(j)
                ipasses = [(xq, wi[:, 0]), (xq, wi[:, 1]), (xd, wi[:, 0])]
                bss = [ds(h * NBH, NBH) for h in range(NH)]
                sigs_h = [[], []]
                pres = [psum_pre.tile([P, NBH], F32, tag="pre",
                                      name=f"pre{j}h{h}")
                        for h in range(NH)]
                if paired:
                    # both batch halves interleaved at each k-pair so a
                    # stationary weight tile feeds two consecutive
                    # matmuls (amortizes the PE weight load)
                    for g, stag in enumerate(("si", "sf", "so")):
                        gps = [psum.tile([P, NBH], F32, tag="gate",
                                         name=f"gp{g}h{h}")
                               for h in range(NH)]
                        for np_, (mv, wt) in enumerate(gp(g)):
                            for t in range(KD2):
                                for h in range(NH):
                                    nc.tensor.matmul(
                                        gps[h][:], wt[:, t],
                                        mv[:, ds(2 * t, 2), bss[h]],
                                        start=(np_ == 0 and t == 0),
                                        stop=(np_ == 2 and t == KD2 - 1),
                                        perf_mode=PM.DoubleRow)
                        for h in range(NH):
                            s = ew.tile([P, NBH], F32, tag=stag,
                                        name=f"s{g}h{h}")
                            nc.scalar.activation(s[:], gps[h][:], AF.Sigmoid,
                                                 scale=INV_PSUM)
                            sigs_h[h].append(s)
                    for np_, (mv, wt) in enumerate(ipasses):
                        for t in range(KD2):
                            for h in range(NH):
                                nc.tensor.matmul(
                                    pres[h][:], wt[:, t],
                                    mv[:, ds(2 * t, 2), bss[h]],
                                    start=(np_ == 0 and t == 0), stop=False,
                                    perf_mode=PM.DoubleRow)
                else:
                    # startup variant: per half, run (g0, g1) in the two
                    # gate banks, then (g2, pre) interleaved - keeps
                    # pace with the streaming x half-tiles
                    for h in range(NH):
                        g0 = psum.tile([P, NBH], F32, tag="gate",
                                       name=f"g0h{h}")
                        g1 = psum.tile([P, NBH], F32, tag="gate",
                                       name=f"g1h{h}")
                        for np_, (mv, wt) in enumerate(gp(0)):
                            wt1 = gp(1)[np_][1]
                            for t in range(KD2):
                                nc.tensor.matmul(
                                    g0[:], wt[:, t],
                                    mv[:, ds(2 * t, 2), bss[h]],
                                    start=(np_ == 0 and t == 0),
                                    stop=(np_ == 2 and t == KD2 - 1),
                                    perf_mode=PM.DoubleRow)
                                nc.tensor.matmul(
                                    g1[:], wt1[:, t],
                                    mv[:, ds(2 * t, 2), bss[h]],
                                    start=(np_ == 0 and t == 0),
                                    stop=(np_ == 2 and t == KD2 - 1),
                                    perf_mode=PM.DoubleRow)
                        s0 = ew.tile([P, NBH], F32, tag="si",
                                     name=f"sA0h{h}")
                        nc.scalar.activation(s0[:], g0[:], AF.Sigmoid,
                                             scale=INV_PSUM)
                        s1 = ew.tile([P, NBH], F32, tag="sf",
                                     name=f"sA1h{h}")
                        nc.scalar.activation(s1[:], g1[:], AF.Sigmoid,
                                             scale=INV_PSUM)
                        g2 = psum.tile([P, NBH], F32, tag="gate",
                                       name=f"g2h{h}")
                        for np_, (mv, wt) in enumerate(gp(2)):
                            miv, wit = ipasses[np_]
                            for t in range(KD2):
                                nc.tensor.matmul(
                                    g2[:], wt[:, t],
                                    mv[:, ds(2 * t, 2), bss[h]],
                                    start=(np_ == 0 and t == 0),
                                    stop=(np_ == 2 and t == KD2 - 1),
                                    perf_mode=PM.DoubleRow)
                                nc.tensor.matmul(
                                    pres[h][:], wit[:, t],
                                    miv[:, ds(2 * t, 2), bss[h]],
                                    start=(np_ == 0 and t == 0), stop=False,
                                    perf_mode=PM.DoubleRow)
                        s2 = ew.tile([P, NBH], F32, tag="so",
                                     name=f"sA2h{h}")
                        nc.scalar.activation(s2[:], g2[:], AF.Sigmoid,
                                             scale=INV_PSUM)
                        sigs_h[h] += [s0, s1, s2]
                for h in range(NH):
                    # fold 9*(sigmoid(f)*h) here - off the B critical path
                    sf_t = sigs_h[h][1]
                    nc.vector.tensor_tensor(sf_t[:], sf_t[:],
                                            hfv[:, j, bss[h]], op=ALU.mult)
                    nc.vector.tensor_scalar(sf_t[:], sf_t[:], 9.0, None,
                                            op0=ALU.mult)
                for h in range(NH):
                    sig_tiles[(j, h)] = sigs_h[h]
                    pre_tiles[(j, h)] = pres[h]

            # Phase B of unit (j, h): finish the pre accumulation with
            # the reservoir part (needs full h + wr_j), then the
            # elementwise epilogue and the output DMA.
            def emit_B(j):
                wr = wrs[j]
                rpasses = [(hq, wr[:, 0]), (hq, wr[:, 1]), (hd, wr[:, 0])]
                bss = [ds(h * NBH, NBH) for h in range(NH)]
                pres = [pre_tiles.pop((j, h)) for h in range(NH)]
                for h in range(NH):
                    for np_, (mv, wt) in enumerate(rpasses):
                        for t in range(KR2):
                            nc.tensor.matmul(
                                pres[h][:], wt[:, t],
                                mv[:, ds(2 * t, 2), bss[h]],
                                start=False,
                                stop=(np_ == 2 and t == KR2 - 1),
                                perf_mode=PM.DoubleRow)
                for h in range(NH):
                    bs = bss[h]
                    si, sf, so = sig_tiles.pop((j, h))
                    pre = pres[h]
                    m = ew.tile([P, NBH], F32, tag="sf", name=f"m{h}")
                    v = vpool.tile([P, NBH], F32, tag="v", name=f"v{h}")
                    # the very last unit runs its epilogue in two column
                    # chunks so the DVE/ACT chains pipeline into the
                    # kernel tail instead of serializing after it
                    last = (j == NJ - 1 and h == NH - 1)
                    chunks = ([ds(0, NBH // 2), ds(NBH // 2, NBH // 2)]
                              if last else [ds(0, NBH)])
                    for cs in chunks:
                        # si <- tanh(si * pre / 8192)
                        nc.vector.tensor_tensor(si[:, cs], si[:, cs],
                                                pre[:, cs], op=ALU.mult)
                        nc.scalar.activation(si[:, cs], si[:, cs], AF.Tanh,
                                             scale=INV_PSUM)
                        # sf already holds 9*(sigmoid(f)*h) from phase A
                        nc.vector.tensor_tensor(sf[:, cs], sf[:, cs],
                                                si[:, cs], op=ALU.add)
                        # so <- po = so*(9fh + t); new_state = 0.1*po
                        nc.vector.tensor_tensor(so[:, cs], so[:, cs],
                                                sf[:, cs], op=ALU.mult)
                        # spike threshold: v = 0.1*po; v>0.5 -> v-0.5
                        nc.vector.tensor_scalar(m[:, cs], so[:, cs], 5.0,
                                                0.5, op0=ALU.is_gt,
                                                op1=ALU.mult)
                        nc.scalar.activation(v[:, cs], so[:, cs], AF.Copy,
                                             scale=0.1)
                        nc.vector.tensor_tensor(v[:, cs], v[:, cs],
                                                m[:, cs], op=ALU.subtract)
                    if last:
                        half = NBH // 2
                        for c in range(2):
                            nc.sync.dma_start(
                                out_d[j * P:(j + 1) * P,
                                      h * NBH + c * half:
                                      h * NBH + (c + 1) * half],
                                v[:, ds(c * half, half)])
                    else:
                        nc.gpsimd.dma_start(
                            out_d[j * P:(j + 1) * P,
                                  h * NBH:(h + 1) * NBH], v[:])

            # DMA order: wg0 per gate interleaved with the x stream
            # (first gates start ~2.5us in), then h with later blocks'
            # weights interleaved so B0 is never the head blocker.
            wg0 = wts.tile([P, 3, 2, KD2, 2, P], F8, tag="wg")
            wi0 = wts.tile([P, 2, KD2, 2, P], F8, tag="wi")
            nc.sync.dma_start(wg0[:, 0], wg_d[0][:, 0])
            nc.sync.dma_start(xq[:, :, 0:NBH], xq_d[:, :, 0:NBH])
            nc.sync.dma_start(wg0[:, 1], wg_d[0][:, 1])
            nc.sync.dma_start(xd[:, :, 0:NBH], xd_d[:, :, 0:NBH])
            nc.sync.dma_start(wg0[:, 2], wg_d[0][:, 2])
            nc.sync.dma_start(wi0[:], wi_d[0])
            nc.sync.dma_start(xq[:, :, NBH:BC], xq_d[:, :, NBH:BC])
            nc.sync.dma_start(xd[:, :, NBH:BC], xd_d[:, :, NBH:BC])
            wgs.append(wg0)
            wis.append(wi0)
            nc.sync.dma_start(hq[:, :, 0:NBH], hq_d[:, :, 0:NBH])
            nc.sync.dma_start(hd[:, :, 0:NBH], hd_d[:, :, 0:NBH])
            load_wgwi(1)
            nc.sync.dma_start(hq[:, :, NBH:BC], hq_d[:, :, NBH:BC])
            nc.sync.dma_start(hd[:, :, NBH:BC], hd_d[:, :, NBH:BC])
            load_wgwi(2)
            load_wr(0)
            nc.sync.dma_start(hfv[:, 0], hf_d[:, 0])
            nc.sync.dma_start(hfv[:, 1], hf_d[:, 1])

            emit_A(0, paired=False)
            emit_A(1)
            for j in range(NJ):
                if j + 1 < NJ:
                    load_wr(j + 1)
                if j + 3 < NJ:
                    load_wgwi(j + 3)
                if j + 2 < NJ:
                    nc.sync.dma_start(hfv[:, j + 2], hf_d[:, j + 2])
                    emit_A(j + 2)
                emit_B(j)

    nc.compile()
    return nc


def _q8(a):
    return np.ascontiguousarray(a, dtype=np.float32).astype(NP8)


def _qpair(a, s):
    """fp8 main + fp8 residual of s*a. Pow2 s makes the scaling exact."""
    sa = np.ascontiguousarray(a, dtype=np.float32) * s
    q = sa.astype(NP8)
    d = (sa - q.astype(np.float32)).astype(NP8)
    return q, d


def _pack_mov(a8):
    """[Kt*P, BC] -> [P, Kt, BC] partition-major moving layout."""
    kt = a8.shape[0] // P
    return np.ascontiguousarray(a8.reshape(kt, P, -1).transpose(1, 0, 2))


def _pack_stat(a8):
    """[Kt*P, RC] -> [NJ, P, Kt/2, 2, P] DoubleRow stationary layout."""
    kt2 = a8.shape[0] // (2 * P)
    return np.ascontiguousarray(
        a8.reshape(kt2, 2, P, NJ, P).transpose(3, 2, 0, 1, 4))


def _shard(inputs, state, reservoir_weights, input_weights, gate_weights):
    x = np.ascontiguousarray(inputs[:, 0, :], dtype=np.float32)
    h = np.ascontiguousarray(state[:, 0, :R], dtype=np.float32)

    # weight prep depends only on the column group (2 variants across 8
    # cores) - compute once per group and share the arrays
    wsets = {}
    for rg in range(RGROUPS):
        rsl = slice(rg * RC, (rg + 1) * RC)
        osl = slice((1 - rg) * RC, (1 - rg) * RC + RC)  # other half
        wr = np.concatenate([reservoir_weights[rsl, rsl.start:rsl.stop],
                             reservoir_weights[osl, rsl.start:rsl.stop]],
                            axis=0)
        wrq, wrd = _qpair(np.asarray(wr), SW)
        wg_full = np.empty([NJ, P, 3, 2, KD2, 2, P], dtype=NP8)
        for g in range(3):
            blk = gate_weights[g * R + rg * RC:g * R + rg * RC + RC, :]
            wgq, wgd = _qpair(np.asarray(blk).T, SW)
            wg_full[:, :, g, 0] = _pack_stat(wgq)
            wg_full[:, :, g, 1] = _pack_stat(wgd)
        wiq, wid = _qpair(np.asarray(input_weights[rsl, :]).T, SW)
        wi_full = np.stack([_pack_stat(wiq), _pack_stat(wid)], axis=2)
        wr_full = np.stack([_pack_stat(wrq), _pack_stat(wrd)], axis=2)
        wsets[rg] = {"wg": wg_full, "wi": np.ascontiguousarray(wi_full),
                     "wr": np.ascontiguousarray(wr_full)}

    in_maps = []
    for core in range(NCORES):
        d_, rg = divmod(core, RGROUPS)
        bsl = slice(d_ * BC, (d_ + 1) * BC)
        rsl = slice(rg * RC, (rg + 1) * RC)
        osl = slice((1 - rg) * RC, (1 - rg) * RC + RC)
        hT_own = h[bsl, rsl].T
        hT = np.concatenate([hT_own, h[bsl, osl].T], axis=0)
        xq8, xd8 = _qpair(x[bsl].T, SX)
        hq8, hd8 = _qpair(hT, SX)
        in_maps.append({
            "xq": _pack_mov(xq8), "xd": _pack_mov(xd8),
            "hq": _pack_mov(hq8), "hd": _pack_mov(hd8),
            "hf": _pack_mov(np.ascontiguousarray(hT_own, dtype=np.float32)),
            **wsets[rg],
        })
    return in_maps


def _run(inputs, state, reservoir_weights, input_weights, gate_weights,
         trace=False):
    if "nc" not in _cache:
        _cache["nc"] = _build()
    nc = _cache["nc"]
    in_maps = _shard(inputs, state, reservoir_weights, input_weights,
                     gate_weights)
    res = run_bass_kernel_spmd(nc, in_maps, core_ids=list(range(NCORES)),
                               trace=trace)
    out = np.zeros((B, 1, MAXR), dtype=np.float32)
    for core in range(NCORES):
        d_, rg = divmod(core, RGROUPS)
        out[d_ * BC:(d_ + 1) * BC, 0, rg * RC:(rg + 1) * RC] = \
            res.results[core]["out"].T
    return out, res


def kernel(inputs, state, reservoir_weights, input_weights, gate_weights):
    out, _ = _run(inputs, state, reservoir_weights, input_weights,
                  gate_weights)
    return out


# revision 26
# speedup vs baseline: 1.3320x; 1.0307x over previous
"""Trainium2 Bass kernel for AdaptiveGatedSLNNStep.

Reference computation (B=4096, D=1024, R=2048, MAXR=4096):
    x  = inputs[:, 0, :]                  # [B, D]
    h  = state[:, 0, :R]                  # [B, R]
    ip = x @ Wi[:R, :].T                  # [B, R]
    rp = h @ Wr[:R, :R]                   # [B, R]
    g  = sigmoid(x @ Wg[:3R, :].T)        # [B, 3R] -> i, f, o
    ns = 0.9*(f*h) + 0.1*tanh(i*(ip+rp))
    ns = o * ns
    ns = where(ns > 0.5, ns - 0.5, ns)
    out = pad(ns, [B, 1, MAXR])

Sharding: 8 cores = 4 batch groups x 2 reservoir-column groups; no
collectives (output blocks are disjoint). Each core computes its
[1024, 1024] block of ns in FEATURE-MAJOR layout: out[r, b].

All matmuls run as fp8 e4m3 in DoubleRow perf mode (two contraction
rows per PE pass), with a 3-pass residual-correction scheme that
recovers ~bf16 accuracy at 0.75x the fp32r cycle count:
    A@B ~= Q(A)Q(B) + Q(dA)Q(B) + Q(A)Q(dB),   dA = A - Q(A)
Operands are pre-scaled by powers of two on the host (x,h by 16,
weights by 512) so every pass shares one PSUM scale (8192), folded
into the sigmoid/tanh activation scale for free. The elementwise
f*h term reads a separate exact fp32 copy of h (fp8 h there would
dominate the error via spike-threshold flips).

h^T rows are permuted on the host so this core's own RC-slice comes
first; wr's contraction rows are permuted identically. The host
transposes per-core outputs while assembling the padded result.
"""

import numpy as np
import ml_dtypes

import concourse.bass as bass
import concourse.mybir as mybir
import concourse.tile as tile
from concourse import bacc
from concourse.bass import ds
from concourse.bass_utils import run_bass_kernel_spmd

F32 = mybir.dt.float32
F16 = mybir.dt.float16
F8 = mybir.dt.float8e4
NP8 = ml_dtypes.float8_e4m3
AF = mybir.ActivationFunctionType
ALU = mybir.AluOpType
PM = mybir.MatmulPerfMode

B = 4096          # global batch
D = 1024          # input dim
R = 2048          # reservoir dim
MAXR = 4096       # padded reservoir dim
NCORES = 8
DGROUPS = 4       # batch groups
RGROUPS = 2       # reservoir column groups
BC = B // DGROUPS     # 1024 batch rows per core
RC = R // RGROUPS     # 1024 reservoir rows (output features) per core
P = 128               # partitions
NJ = RC // P          # 8 reservoir row blocks per core
NBH = 512             # batch columns per matmul (moving operand)
NH = BC // NBH        # 2 batch halves
KD = D // P           # 8  contraction tiles over D
KD2 = KD // 2         # 4  DoubleRow k-pairs over D
KR = R // P           # 16 contraction tiles over R
KR2 = KR // 2         # 8  DoubleRow k-pairs over R

SX = np.float32(16.0)    # x/h fp8 pre-scale
SW = np.float32(512.0)   # weight fp8 pre-scale
INV_PSUM = float(1.0 / (float(SX) * float(SW)))   # 1/8192

_cache = {}


def _build():
    nc = bacc.Bacc("TRN2", target_bir_lowering=False, debug=False,
                   num_devices=NCORES)

    # q/d pairs: main fp8 quantization and its fp8-quantized residual
    xq_d = nc.dram_tensor("xq", [P, KD, BC], F8, kind="ExternalInput")
    xd_d = nc.dram_tensor("xd", [P, KD, BC], F8, kind="ExternalInput")
    hq_d = nc.dram_tensor("hq", [P, KR, BC], F8, kind="ExternalInput")
    hd_d = nc.dram_tensor("hd", [P, KR, BC], F8, kind="ExternalInput")
    # exact h (own half, feature-major) for the elementwise f*h term
    hf_d = nc.dram_tensor("hf", [P, NJ, BC], F16, kind="ExternalInput")
    # weights packed per reservoir block, partition-major, with the q/d
    # variants adjacent and contraction pre-grouped into DoubleRow pairs:
    # wg[j, p, g, qd, t, i, m] = Wsc[(2t+i)*128+p, j*128+m]
    wg_d = nc.dram_tensor("wg", [NJ, P, 3, 2, KD2, 2, P], F8,
                          kind="ExternalInput")
    wi_d = nc.dram_tensor("wi", [NJ, P, 2, KD2, 2, P], F8,
                          kind="ExternalInput")
    wr_d = nc.dram_tensor("wr", [NJ, P, 2, KR2, 2, P], F8,
                          kind="ExternalInput")
    out_d = nc.dram_tensor("out", [RC, BC], F16, kind="ExternalOutput")

    with tile.TileContext(nc) as tc:
        with (
            tc.tile_pool(name="acts", bufs=1) as acts,
            tc.tile_pool(name="wts", bufs=3) as wts,
            tc.tile_pool(name="wrp", bufs=2) as wrp,
            tc.tile_pool(name="ew", bufs=6) as ew,
            tc.tile_pool(name="vpool", bufs=2) as vpool,
            tc.tile_pool(name="psum", bufs=2, space="PSUM") as psum,
            tc.tile_pool(name="psum_pre", bufs=6, space="PSUM") as psum_pre,
        ):
            # resident moving operands
            xq = acts.tile([P, KD, BC], F8, tag="xq")
            xd = acts.tile([P, KD, BC], F8, tag="xd")
            hq = acts.tile([P, KR, BC], F8, tag="hq")
            hd = acts.tile([P, KR, BC], F8, tag="hd")
            hfv = acts.tile([P, NJ, BC], F16, tag="hf")

            # PE clock warmup: the clock gate only releases full rate
            # after ~3us of sustained PE activity, and the first real
            # matmul can't start until its weights arrive. Burn that
            # window on dummy matmuls over a zeroed tile.
            warm = acts.tile([P, P], F32, tag="warm")
            wact = acts.tile([P, 4], F32, tag="wact")
            nc.gpsimd.memset(warm[:], 0.0)
            # preload the ACT function tables (1.3us each) during the
            # head DMA window so the first real sigmoid doesn't pay
            nc.scalar.activation(wact[:], warm[:, 0:4], AF.Sigmoid)
            nc.scalar.activation(wact[:], warm[:, 0:4], AF.Tanh)
            # sized so warmup ends just as the first gate weights +
            # x half land (~2.8us): the PE stays continuously busy into
            # the real matmuls and finishes ramping under real work
            wpsum = psum.tile([P, P], F32, tag="gate", name="warmp")
            NWARM = 3
            for w in range(NWARM):
                nc.tensor.matmul(wpsum[:], warm[:], warm[:],
                                 start=(w == 0), stop=(w == NWARM - 1))
            nc.scalar.activation(warm[:], wpsum[:], AF.Copy, scale=0.0)

            wgs, wis, wrs = [], [], []

            def load_wgwi(j, split=False):
                wg = wts.tile([P, 3, 2, KD2, 2, P], F8, tag="wg")
                if split:
                    # g0 lands first so phase A can start while the rest
                    # of the block's weights stream in
                    nc.sync.dma_start(wg[:, 0], wg_d[j][:, 0])
                    nc.sync.dma_start(wg[:, 1:], wg_d[j][:, 1:])
                else:
                    nc.sync.dma_start(wg[:], wg_d[j])
                wi = wts.tile([P, 2, KD2, 2, P], F8, tag="wi")
                nc.sync.dma_start(wi[:], wi_d[j])
                wgs.append(wg)
                wis.append(wi)

            def load_wr(j):
                wr = wrp.tile([P, 2, KR2, 2, P], F8, tag="wr")
                nc.sync.dma_start(wr[:], wr_d[j])
                wrs.append(wr)

            # The three correction passes per matmul group: (moving
            # operand, qd index of the stationary operand). xd last so
            # startup DMA has extra slack for the residual tensors.
            def gate_passes(j):
                wg = wgs[j]
                return lambda g: [(xq, wg[:, g, 0]), (xq, wg[:, g, 1]),
                                  (xd, wg[:, g, 0])]

            # Phase A of unit (j, h): three gate matmul groups, each
            # drained to SBUF by a sigmoid immediately (2 rotating PSUM
            # banks), plus the input-part matmuls left OPEN in a pre
            # bank. Only needs x + wg_j + wi_j.
            sig_tiles = {}
            pre_tiles = {}

            def emit_A(j, paired=True):
                wg, wi = wgs[j], wis[j]
                gp = gate_passes(j)
                ipasses = [(xq, wi[:, 0]), (xq, wi[:, 1]), (xd, wi[:, 0])]
                bss = [ds(h * NBH, NBH) for h in range(NH)]
                sigs_h = [[], []]
                pres = [psum_pre.tile([P, NBH], F32, tag="pre",
                                      name=f"pre{j}h{h}")
                        for h in range(NH)]
                if paired:
                    # both batch halves interleaved at each k-pair so a
                    # stationary weight tile feeds two consecutive
                    # matmuls (amortizes the PE weight load)
                    for g, stag in enumerate(("si", "sf", "so")):
                        gps = [psum.tile([P, NBH], F32, tag="gate",
                                         name=f"gp{g}h{h}")
                               for h in range(NH)]
                        for np_, (mv, wt) in enumerate(gp(g)):
                            for t in range(KD2):
                                for h in range(NH):
                                    nc.tensor.matmul(
                                        gps[h][:], wt[:, t],
                                        mv[:, ds(2 * t, 2), bss[h]],
                                        start=(np_ == 0 and t == 0),
                                        stop=(np_ == 2 and t == KD2 - 1),
                                        perf_mode=PM.DoubleRow)
                        for h in range(NH):
                            s = ew.tile([P, NBH], F16, tag=stag,
                                        name=f"s{g}h{h}")
                            nc.scalar.activation(s[:], gps[h][:], AF.Sigmoid,
                                                 scale=INV_PSUM)
                            sigs_h[h].append(s)
                    for np_, (mv, wt) in enumerate(ipasses):
                        for t in range(KD2):
                            for h in range(NH):
                                nc.tensor.matmul(
                                    pres[h][:], wt[:, t],
                                    mv[:, ds(2 * t, 2), bss[h]],
                                    start=(np_ == 0 and t == 0), stop=False,
                                    perf_mode=PM.DoubleRow)
                else:
                    # startup variant: per half, run g0 alone (its
                    # weights land first), then g1, then (g2, pre)
                    # interleaved - matches the head DMA stream order
                    for h in range(NH):
                        tags = ("si", "sf")
                        for g in range(2):
                            gb = psum.tile([P, NBH], F32, tag="gate",
                                           name=f"g{g}h{h}")
                            for np_, (mv, wt) in enumerate(gp(g)):
                                for t in range(KD2):
                                    nc.tensor.matmul(
                                        gb[:], wt[:, t],
                                        mv[:, ds(2 * t, 2), bss[h]],
                                        start=(np_ == 0 and t == 0),
                                        stop=(np_ == 2 and t == KD2 - 1),
                                        perf_mode=PM.DoubleRow)
                            s = ew.tile([P, NBH], F16, tag=tags[g],
                                        name=f"sA{g}h{h}")
                            nc.scalar.activation(s[:], gb[:], AF.Sigmoid,
                                                 scale=INV_PSUM)
                            sigs_h[h].append(s)
                        g2 = psum.tile([P, NBH], F32, tag="gate",
                                       name=f"g2h{h}")
                        for np_, (mv, wt) in enumerate(gp(2)):
                            miv, wit = ipasses[np_]
                            for t in range(KD2):
                                nc.tensor.matmul(
                                    g2[:], wt[:, t],
                                    mv[:, ds(2 * t, 2), bss[h]],
                                    start=(np_ == 0 and t == 0),
                                    stop=(np_ == 2 and t == KD2 - 1),
                                    perf_mode=PM.DoubleRow)
                                nc.tensor.matmul(
                                    pres[h][:], wit[:, t],
                                    miv[:, ds(2 * t, 2), bss[h]],
                                    start=(np_ == 0 and t == 0), stop=False,
                                    perf_mode=PM.DoubleRow)
                        s2 = ew.tile([P, NBH], F16, tag="so",
                                     name=f"sA2h{h}")
                        nc.scalar.activation(s2[:], g2[:], AF.Sigmoid,
                                             scale=INV_PSUM)
                        sigs_h[h].append(s2)
                for h in range(NH):
                    # fold sigmoid(f)*(9h) here - off the B critical
                    # path; hfv holds 9*h from the host. Runs on the
                    # otherwise-idle GPSIMD engine to keep DVE free for
                    # the epilogue chains.
                    sf_t = sigs_h[h][1]
                    nc.gpsimd.tensor_tensor(sf_t[:], sf_t[:],
                                            hfv[:, j, bss[h]], op=ALU.mult)
                for h in range(NH):
                    sig_tiles[(j, h)] = sigs_h[h]
                    pre_tiles[(j, h)] = pres[h]

            # Phase B of unit (j, h): finish the pre accumulation with
            # the reservoir part (needs full h + wr_j), then the
            # elementwise epilogue and the output DMA.
            def emit_B(j):
                wr = wrs[j]
                rpasses = [(hq, wr[:, 0]), (hq, wr[:, 1]), (hd, wr[:, 0])]
                bss = [ds(h * NBH, NBH) for h in range(NH)]
                pres = [pre_tiles.pop((j, h)) for h in range(NH)]
                # per-half pipelining: half h's epilogue chains run on
                # DVE/ACT underneath half h+1's reservoir matmuls. The
                # very last unit is additionally column-split with the
                # reservoir matmuls themselves chunked, so each chunk's
                # epilogue hides under the next chunk's matmuls and only
                # one short chain trails the final matmul.
                for h in range(NH):
                    for np_, (mv, wt) in enumerate(rpasses):
                        for t in range(KR2):
                            nc.tensor.matmul(
                                pres[h][:], wt[:, t],
                                mv[:, ds(2 * t, 2), bss[h]],
                                start=False,
                                stop=(np_ == 2 and t == KR2 - 1),
                                perf_mode=PM.DoubleRow)
                    si, sf, so = sig_tiles.pop((j, h))
                    pre = pres[h]
                    m = ew.tile([P, NBH], F16, tag="sf", name=f"m{h}")
                    v = vpool.tile([P, NBH], F16, tag="v", name=f"v{h}")
                    nechunk = {(NJ - 1, 1): 4, (NJ - 1, 0): 2,
                               (NJ - 2, 1): 2}.get((j, h), 1)
                    ecw = NBH // nechunk
                    for c in range(nechunk):
                        _epi(j, h, c * ecw, ecw, si, sf, so, pre, m, v,
                             nechunk > 1)
                    if nechunk == 1:
                        nc.gpsimd.dma_start(
                            out_d[j * P:(j + 1) * P,
                                  h * NBH:(h + 1) * NBH], v[:])

            def _epi(j, h, c0, cw, si, sf, so, pre, m, v, dma_now):
                cs = ds(c0, cw)
                # si <- tanh(si * pre / 8192)
                nc.vector.tensor_tensor(si[:, cs], si[:, cs],
                                        pre[:, cs], op=ALU.mult)
                nc.scalar.activation(si[:, cs], si[:, cs], AF.Tanh,
                                     scale=INV_PSUM)
                # sf already holds sigmoid(f)*9h from phase A
                nc.vector.tensor_tensor(sf[:, cs], sf[:, cs],
                                        si[:, cs], op=ALU.add)
                # so <- po = so*(9fh + t); out = po - 5*(po>5)
                # (host scales by 0.1: 0.1*po - 0.5*(po>5))
                nc.vector.tensor_tensor(so[:, cs], so[:, cs],
                                        sf[:, cs], op=ALU.mult)
                nc.vector.tensor_scalar(m[:, cs], so[:, cs], 5.0,
                                        5.0, op0=ALU.is_gt, op1=ALU.mult)
                nc.vector.tensor_tensor(v[:, cs], so[:, cs],
                                        m[:, cs], op=ALU.subtract)
                if dma_now:
                    # alternate issue queues: each hwdge engine pays
                    # ~600ns SEQ per dma_start, so splitting the tail
                    # chunk stores across SP/ACT halves the issue stall
                    eng = nc.scalar if (c0 // cw) % 2 else nc.sync
                    eng.dma_start(
                        out_d[j * P:(j + 1) * P,
                              h * NBH + c0:h * NBH + c0 + cw],
                        v[:, cs])

            # DMA order: wg0 per gate interleaved with the x stream
            # (first gates start ~2.5us in), then h with later blocks'
            # weights interleaved so B0 is never the head blocker.
            wg0 = wts.tile([P, 3, 2, KD2, 2, P], F8, tag="wg")
            wi0 = wts.tile([P, 2, KD2, 2, P], F8, tag="wi")
            nc.sync.dma_start(wg0[:, 0], wg_d[0][:, 0])
            nc.sync.dma_start(xq[:, 0:2, 0:NBH], xq_d[:, 0:2, 0:NBH])
            nc.sync.dma_start(xq[:, 2:, 0:NBH], xq_d[:, 2:, 0:NBH])
            nc.sync.dma_start(xd[:, :, 0:NBH], xd_d[:, :, 0:NBH])
            nc.sync.dma_start(wg0[:, 1], wg_d[0][:, 1])
            nc.sync.dma_start(wg0[:, 2], wg_d[0][:, 2])
            nc.sync.dma_start(wi0[:], wi_d[0])
            nc.sync.dma_start(xq[:, :, NBH:BC], xq_d[:, :, NBH:BC])
            nc.sync.dma_start(xd[:, :, NBH:BC], xd_d[:, :, NBH:BC])
            wgs.append(wg0)
            wis.append(wi0)
            load_wgwi(1, split=True)
            load_wgwi(2, split=True)
            nc.sync.dma_start(hq[:, :, 0:NBH], hq_d[:, :, 0:NBH])
            nc.sync.dma_start(hd[:, :, 0:NBH], hd_d[:, :, 0:NBH])
            load_wr(0)
            nc.sync.dma_start(hq[:, :, NBH:BC], hq_d[:, :, NBH:BC])
            nc.sync.dma_start(hd[:, :, NBH:BC], hd_d[:, :, NBH:BC])
            nc.sync.dma_start(hfv[:, 0], hf_d[:, 0])
            nc.sync.dma_start(hfv[:, 1], hf_d[:, 1])

            emit_A(0, paired=False)
            emit_A(1)
            for j in range(NJ):
                if j + 1 < NJ:
                    load_wr(j + 1)
                if j + 3 < NJ:
                    load_wgwi(j + 3)
                if j + 2 < NJ:
                    nc.sync.dma_start(hfv[:, j + 2], hf_d[:, j + 2])
                    emit_A(j + 2)
                emit_B(j)

    nc.compile()
    return nc


def _q8(a):
    return np.ascontiguousarray(a, dtype=np.float32).astype(NP8)


def _qpair(a, s):
    """fp8 main + fp8 residual of s*a. Pow2 s makes the scaling exact."""
    sa = np.ascontiguousarray(a, dtype=np.float32) * s
    q = sa.astype(NP8)
    d = (sa - q.astype(np.float32)).astype(NP8)
    return q, d


def _pack_mov(a8):
    """[Kt*P, BC] -> [P, Kt, BC] partition-major moving layout."""
    kt = a8.shape[0] // P
    return np.ascontiguousarray(a8.reshape(kt, P, -1).transpose(1, 0, 2))


def _pack_stat(a8):
    """[Kt*P, RC] -> [NJ, P, Kt/2, 2, P] DoubleRow stationary layout."""
    kt2 = a8.shape[0] // (2 * P)
    return np.ascontiguousarray(
        a8.reshape(kt2, 2, P, NJ, P).transpose(3, 2, 0, 1, 4))


def _shard(inputs, state, reservoir_weights, input_weights, gate_weights):
    x = np.ascontiguousarray(inputs[:, 0, :], dtype=np.float32)
    h = np.ascontiguousarray(state[:, 0, :R], dtype=np.float32)

    # weight prep depends only on the column group (2 variants across 8
    # cores) - compute once per group and share the arrays
    wsets = {}
    for rg in range(RGROUPS):
        rsl = slice(rg * RC, (rg + 1) * RC)
        osl = slice((1 - rg) * RC, (1 - rg) * RC + RC)  # other half
        wr = np.concatenate([reservoir_weights[rsl, rsl.start:rsl.stop],
                             reservoir_weights[osl, rsl.start:rsl.stop]],
                            axis=0)
        wrq, wrd = _qpair(np.asarray(wr), SW)
        wg_full = np.empty([NJ, P, 3, 2, KD2, 2, P], dtype=NP8)
        for g in range(3):
            blk = gate_weights[g * R + rg * RC:g * R + rg * RC + RC, :]
            wgq, wgd = _qpair(np.asarray(blk).T, SW)
            wg_full[:, :, g, 0] = _pack_stat(wgq)
            wg_full[:, :, g, 1] = _pack_stat(wgd)
        wiq, wid = _qpair(np.asarray(input_weights[rsl, :]).T, SW)
        wi_full = np.stack([_pack_stat(wiq), _pack_stat(wid)], axis=2)
        wr_full = np.stack([_pack_stat(wrq), _pack_stat(wrd)], axis=2)
        wsets[rg] = {"wg": wg_full, "wi": np.ascontiguousarray(wi_full),
                     "wr": np.ascontiguousarray(wr_full)}

    in_maps = []
    for core in range(NCORES):
        d_, rg = divmod(core, RGROUPS)
        bsl = slice(d_ * BC, (d_ + 1) * BC)
        rsl = slice(rg * RC, (rg + 1) * RC)
        osl = slice((1 - rg) * RC, (1 - rg) * RC + RC)
        hT_own = h[bsl, rsl].T
        hT = np.concatenate([hT_own, h[bsl, osl].T], axis=0)
        xq8, xd8 = _qpair(x[bsl].T, SX)
        hq8, hd8 = _qpair(hT, SX)
        in_maps.append({
            "xq": _pack_mov(xq8), "xd": _pack_mov(xd8),
            "hq": _pack_mov(hq8), "hd": _pack_mov(hd8),
            "hf": _pack_mov((np.float32(9.0)
                             * np.asarray(hT_own, dtype=np.float32))
                            .astype(np.float16)),
            **wsets[rg],
        })
    return in_maps


def _run(inputs, state, reservoir_weights, input_weights, gate_weights,
         trace=False):
    if "nc" not in _cache:
        _cache["nc"] = _build()
    nc = _cache["nc"]
    in_maps = _shard(inputs, state, reservoir_weights, input_weights,
                     gate_weights)
    res = run_bass_kernel_spmd(nc, in_maps, core_ids=list(range(NCORES)),
                               trace=trace)
    out = np.zeros((B, 1, MAXR), dtype=np.float32)
    for core in range(NCORES):
        d_, rg = divmod(core, RGROUPS)
        # device emits fp16 po - 5*(po>5); the 0.1 leak scale lands here
        out[d_ * BC:(d_ + 1) * BC, 0, rg * RC:(rg + 1) * RC] = \
            np.float32(0.1) * res.results[core]["out"].T.astype(np.float32)
    return out, res


def kernel(inputs, state, reservoir_weights, input_weights, gate_weights):
    out, _ = _run(inputs, state, reservoir_weights, input_weights,
                  gate_weights)
    return out
